# revision 34
# baseline (speedup 1.0000x reference)
"""Multi-head self-attention (B=2, N=2048, C=1024, H=16) on 8 TRN2 NeuronCores.

Sharding: data-parallel over batch (2) x tensor-parallel over heads (16/4=4 groups).
Core c handles batch b=c//4 and heads [4*(c%4), 4*(c%4)+4).

v3f (fp16 everywhere):
  - Inputs stream as a few wide strided DMAs ordered by need: q01/k01
    weights + the first quarter of x gate the first score (~16us), then V
    weights, the remaining x quarters, q23/k23 weights, w_out.
  - Wave-1 computes only q01[ic0] + k01[ic0] (the first-score gate) on one
    PSUM pair. Everything else in the QKV projection (16 V tiles, k01[ic1-3],
    q01[ic1-3], q23, k23) is split into small matmul units popped into PE
    idle slots under the ACT-saturated attention stream: 3 units/slot during
    chunk (p0,ic0) with deadlines v[nt] <= slot nt, k01[ic] <= slot 4*ic-1,
    then 1 unit/slot.
  - Attention per head pair: S^T = K^T.T @ Q^T, two heads packed in disjoint
    PE row groups (concurrent); P^T = exp(S*scale) on ACT (the pacing
    engine); O_aug^T = [V|1]^T @ P^T accumulated over key tiles (ones-column
    = softmax sums). The scores matmul for slot t+1 is emitted BEFORE the AV
    matmuls of slot t, so at chunk boundaries the next chunk's first exp
    follows the previous one back-to-back instead of waiting out the AV
    pipeline refill.
  - Normalize: sums row -> partition 0 via small SBUF DMA,
    reciprocal_approx_fast, gpsimd partition_broadcast, one DVE multiply per
    head (odd head shifts to partitions 64-127 via SBUF DMA).
  - Out-projection y_groups for chunk ic pop into chunk ic+1's slots; the
    final chunk's groups split their contraction (pair0 | pair1-even |
    pair1-odd) so matmuls overlap the normalize chain, with PSUM->SBUF
    copies alternating between DVE and the now-idle ACT; fp16 output.
Host sums the 4 per-batch partials (head groups) in fp32 and adds b_out.
"""

import contextlib

import numpy as np

import concourse.bass as bass
import concourse.bacc as bacc
import concourse.tile as tile
from concourse import library_config, mybir
from concourse.bass_utils import run_bass_kernel_spmd

B, NSEQ, CDIM, NHEADS, HD = 2, 2048, 1024, 16, 64
NH = 4          # heads per core
NCORES = 8
F32 = mybir.dt.float32
F16 = mybir.dt.float16
EXP = mybir.ActivationFunctionType.Exp
SCALE = HD ** -0.5


def build_program(dbg_probes=False):
    nc = bacc.Bacc("TRN2", target_bir_lowering=False, debug=False)

    xT = nc.dram_tensor("xT", [CDIM, NSEQ], F16, kind="ExternalInput").ap()
    wqkv = nc.dram_tensor("wqkv", [CDIM, 3 * NH * HD], F16, kind="ExternalInput").ap()
    wout = nc.dram_tensor("wout", [NH * HD, CDIM], F16, kind="ExternalInput").ap()
    y = nc.dram_tensor("y", [NSEQ, CDIM], F16, kind="ExternalOutput").ap()

    with tile.TileContext(nc) as tc:
        emit(nc, tc, xT, wqkv, wout, y)

    nc.compile()
    return nc


def emit(nc, tc, xT, wqkv, wout, y):
    ctx = contextlib.ExitStack()
    with ctx:
        const = ctx.enter_context(tc.tile_pool(name="const", bufs=1))

        # ---- persistent SBUF tensors ----
        wqkv_sb = const.tile([128, 8, 3 * NH * HD], F16)    # [p, ctile, 768]
        wout_sb = const.tile([128, 2, CDIM], F16)           # [p, ktile, 1024]
        xT_sb = const.tile([128, 8, NSEQ], F16)             # [p, ctile, 2048]
        qk_sb = const.tile([128, 4, NSEQ], F16)             # dim1: q01,q23,k01,k23
        v_aug = const.tile([128, 16, NH, HD + 1], F16)      # [p, ntile, head, V|1]
        o_sb = const.tile([128, 2, NSEQ], F16)              # normalized O^T, pairs
        r1t = const.tile([128, 4, 512], F32)                # sums/recip rows
        ones_b = const.tile([128, HD], F32)                 # fast-path bcast lhsT
        wout_lo = const.tile([64, CDIM], F16)               # wout pair1 odd rows @p0

        nc.gpsimd.load_library(library_config.attn)
        nc.vector.memset(v_aug[:, :, :, HD:HD + 1], 1.0)
        nc.vector.memset(ones_b, 1.0)

        # PSUM: 8 banks. sb (2 x [128,1024] = 4): wave-1 pair then scores
        # ping/pong. qk, vp: rolling accumulators for deferred QKV units and
        # out-proj psy. o0, o1: AV accumulators.
        with tc.tile_pool(name="pP", bufs=20) as pP, \
             tc.tile_pool(name="oup", bufs=2) as oup, \
             tc.tile_pool(name="rbc", bufs=4) as rbc, \
             tc.tile_pool(name="shf", bufs=2) as shf, \
             tc.tile_pool(name="yb", bufs=4) as yb, \
             tc.tile_pool(name="psm", bufs=1, space="PSUM") as psm:

            xT_t = xT.rearrange("(t p) n -> p t n", p=128)
            wqkv_t = wqkv.rearrange("(t p) f -> p t f", p=128)
            wout_t = wout.rearrange("(t p) f -> p t f", p=128)

            # -------- input DMA, ordered by first consumer -------------------
            # host wqkv col layout: [q01 | k01 | q23 | k23 | v]
            nc.sync.dma_start(wqkv_sb[:, :, 0:256], wqkv_t[:, :, 0:256])
            nc.sync.dma_start(xT_sb[:, 0:4, 0:512], xT_t[:, 0:4, 0:512])
            nc.sync.dma_start(xT_sb[:, 4:8, 0:512], xT_t[:, 4:8, 0:512])
            nc.sync.dma_start(wqkv_sb[:, :, 512:768], wqkv_t[:, :, 512:768])
            for cc in (1, 2, 3):
                nc.sync.dma_start(xT_sb[:, :, cc * 512:(cc + 1) * 512],
                                  xT_t[:, :, cc * 512:(cc + 1) * 512])
            nc.sync.dma_start(wqkv_sb[:, :, 256:512], wqkv_t[:, :, 256:512])
            for kt in range(2):
                nc.sync.dma_start(wout_sb[:, kt, :], wout_t[:, kt, :])
            nc.sync.dma_start(wout_lo, wout[192:256, :])

            TB = {"qk": 1, "vp": 1, "sb": 2, "o0": 1, "o1": 1}

            def ptile(tag, shape=(128, 512)):
                return psm.tile(list(shape), F32, tag=tag, bufs=TB[tag],
                                name=tag)

            # wqkv_sb column offset per feature group (host layout order)
            FT_COL = {0: 0, 2: 128, 1: 256, 3: 384}

            def qk_mms(ps, ft, ic, cts):
                c0 = FT_COL[ft]
                for ct in cts:
                    nc.tensor.matmul(
                        ps,
                        wqkv_sb[:, ct, c0:c0 + 128],
                        xT_sb[:, ct, ic * 512:(ic + 1) * 512],
                        start=(ct == 0), stop=(ct == 7),
                    )

            def v_mms(ps, nt, cts):
                for ct in cts:
                    nc.tensor.matmul(
                        ps[:, 0:256],
                        xT_sb[:, ct, nt * 128:(nt + 1) * 128],
                        wqkv_sb[:, ct, 512:768],
                        start=(ct == 0), stop=(ct == 7),
                    )

            def qk_evac(ps, ft, ic):
                nc.vector.tensor_copy(
                    qk_sb[:, ft, ic * 512:(ic + 1) * 512], ps)

            def v_evac(ps, nt):
                nc.vector.tensor_copy(v_aug[:, nt, :, 0:HD], ps[:, 0:256])

            # ---------------- QKV wave 1: the first-score gate ---------------
            sbA = ptile("sb", (128, 1024))
            for ct in range(8):
                qk_mms(sbA[:, 0:512], 0, 0, [ct])
                qk_mms(sbA[:, 512:1024], 2, 0, [ct])
            qk_evac(sbA[:, 0:512], 0, 0)
            qk_evac(sbA[:, 512:1024], 2, 0)

            # -------- deferred QKV chains as small matmul units --------------
            _tag_state = [0]

            def u_tag():
                _tag_state[0] ^= 1
                return "qk" if _tag_state[0] else "vp"

            def chain_units(kind, a):
                holder = {}
                parts = ([[0, 1], [2, 3], [4, 5], [6, 7]] if kind == "qk"
                         else [[0, 1, 2, 3], [4, 5, 6, 7]])

                def mk(cts, last):
                    def f():
                        if "tag" not in holder:
                            holder["tag"] = u_tag()
                        if cts[0] == 0:
                            holder["ps"] = ptile(holder["tag"])
                        ps = holder["ps"]
                        if kind == "qk":
                            qk_mms(ps, a[0], a[1], cts)
                            if last:
                                qk_evac(ps, a[0], a[1])
                        else:
                            v_mms(ps, a, cts)
                            if last:
                                v_evac(ps, a)
                    return f
                return [mk(cts, i == len(parts) - 1)
                        for i, cts in enumerate(parts)]

            V = {nt: chain_units("v", nt) for nt in range(16)}
            K1 = {ic: chain_units("qk", (2, ic)) for ic in (1, 2, 3)}
            Q01 = {ic: chain_units("qk", (0, ic)) for ic in (1, 2, 3)}
            Q23 = [chain_units("qk", (1, ic)) for ic in range(4)]
            K23 = [chain_units("qk", (3, ic)) for ic in range(4)]

            # Per-chunk pop queues. AV for chunk c runs one chunk later (pt
            # tiles are buffered), so (0,0) is scores+exp only and absorbs
            # k01[ic] (needed by slot 4*ic) plus half the V tiles; V[nt] is
            # only consumed once AV(0,0) runs during (0,1), slot nt.
            q00 = (K1[1][0:2] + [V[0][0], V[0][1]] + K1[1][2:4]
                   + [V[1][0], V[1][1]] + K1[2][0:2] + [V[2][0], V[2][1]]
                   + K1[2][2:4] + [V[3][0], V[3][1]] + K1[3][0:2]
                   + [V[4][0], V[4][1]] + K1[3][2:4] + [V[5][0], V[5][1]]
                   + Q01[1] + [V[6][0], V[6][1], V[7][0], V[7][1]])
            q01_ = ([u for nt in range(8, 16) for u in V[nt]] + Q01[2])
            q02 = (Q01[3] + [u for ch in Q23 for u in ch])
            q03 = [u for ch in K23 for u in ch]
            POPQ = {(0, 0): (q00, lambda jt: 2),
                    (0, 1): (q01_, lambda jt: 2 if jt < 4 else 1),
                    (0, 2): (q02, lambda jt: 2 if jt < 4 else 1),
                    (0, 3): (q03, lambda jt: 1)}

            yq = []  # deferred out-projection groups

            def y_group(it, fc):
                psy = ptile(u_tag())
                for pp in range(2):
                    nc.tensor.matmul(
                        psy,
                        o_sb[:, pp, it * 128:(it + 1) * 128],
                        wout_sb[:, pp, fc * 512:(fc + 1) * 512],
                        start=(pp == 0), stop=(pp == 1),
                    )
                y_sb = yb.tile([128, 512], F16, tag="ysb", name="ysbt")
                nc.vector.tensor_copy(y_sb, psy)
                nc.sync.dma_start(
                    y[it * 128:(it + 1) * 128, fc * 512:(fc + 1) * 512], y_sb)

            # final-chunk out-projection. The pair0 matmuls are emitted before
            # the normalize chain (keeps PE warm); the pair1-even half (K=64,
            # base 0) joins after the even mul; the pair1-odd half contracts
            # straight from the pre-shift `tmp` tile against wout_lo (both
            # base 0 - same row group as pair1-even, so the PE serializes
            # them; a 0/64-split pair would drain into one PSUM bank
            # concurrently = collision). No shift DMA on the critical path.
            DRAIN_GROUPS = [(12 + k // 2, k % 2) for k in range(8)]

            def y_drain_open(ic):
                psys = []
                for it, fc in DRAIN_GROUPS[:2]:
                    psy = ptile(u_tag())
                    nc.tensor.matmul(
                        psy, o_sb[:, 0, it * 128:(it + 1) * 128],
                        wout_sb[:, 0, fc * 512:(fc + 1) * 512],
                        start=True, stop=False)
                    psys.append(psy)
                return psys

            def y_drain_close(ic, psys, tmp):
                i0 = ic * 512

                def fin(g, it, fc, psy):
                    i1, f1 = it * 128, fc * 512
                    nc.tensor.matmul(
                        psy, o_sb[0:64, 1, i1:i1 + 128],
                        wout_sb[0:64, 1, f1:f1 + 512],
                        start=False, stop=False)
                    nc.tensor.matmul(
                        psy, tmp[:, i1 - i0:i1 - i0 + 128],
                        wout_lo[:, f1:f1 + 512], start=False, stop=True)
                    y_sb = yb.tile([128, 512], F16, tag="ysb", name="ysbt")
                    if g % 2 == 0:
                        nc.vector.tensor_copy(y_sb, psy)
                    else:
                        nc.scalar.copy(y_sb, psy)
                    nc.sync.dma_start(y[i1:i1 + 128, f1:f1 + 512], y_sb)

                for g, (it, fc) in enumerate(DRAIN_GROUPS[:2]):
                    fin(g, it, fc, psys[g])
                for g, (it, fc) in enumerate(DRAIN_GROUPS[2:], start=2):
                    psy = ptile(u_tag())
                    nc.tensor.matmul(
                        psy, o_sb[:, 0, it * 128:(it + 1) * 128],
                        wout_sb[:, 0, fc * 512:(fc + 1) * 512],
                        start=True, stop=False)
                    fin(g, it, fc, psy)

            def normalize(p, ic, po, tags=("o0", "o1"), fast=False):
                i0 = ic * 512
                o_u = [oup.tile([HD + 1, 512], F32, tag=f"ou{e}",
                                name=f"ou{e}") for e in range(2)]
                # e1 chain first: its shift DMA is the longest pole
                nc.vector.tensor_copy(o_u[1], po[1][0:HD + 1, :])
                nc.vector.tensor_copy(o_u[0], po[0][0:HD + 1, :])
                rb = [None, None]
                if fast:
                    # tail path: broadcast the sums row with a small fp32 PE
                    # matmul into the just-freed po bank (PE is idle here and
                    # this keeps HAM warm), then reciprocal on 64 lanes
                    for e in (1, 0):
                        rbp = ptile(tags[e])
                        nc.tensor.matmul(
                            rbp[0:64, :], ones_b[HD:HD + 1, :],
                            o_u[e][HD:HD + 1, :], start=True, stop=True)
                        rb[e] = rbc.tile([64, 512], F32, tag="rb", name="rb")
                        nc.vector.reciprocal_approx_fast(rb[e], rbp[0:64, :])
                else:
                    r1 = [None, None]
                    for e in (1, 0):
                        r0 = r1t[0:1, 2 * e, :]
                        nc.sync.dma_start(r0, o_u[e][HD:HD + 1, :])
                        r1[e] = r1t[0:1, 2 * e + 1, :]
                        nc.vector.reciprocal_approx_fast(r1[e], r0)
                    for e in (1, 0):
                        rb[e] = rbc.tile([64, 512], F32, tag="rb", name="rb")
                        nc.gpsimd.partition_broadcast(rb[e], r1[e])
                tmp = shf.tile([64, 512], F16, tag="tmp")
                nc.vector.tensor_mul(tmp, o_u[1][0:64, :], rb[1])
                if not fast:  # the drain reads tmp directly instead
                    nc.sync.dma_start(o_sb[64:128, p, i0:i0 + 512], tmp)
                nc.vector.tensor_mul(
                    o_sb[0:64, p, i0:i0 + 512], o_u[0][0:64, :], rb[0])
                return tmp

            # ---------------- attention (flat, scores one slot ahead) --------
            def s_group(p, ic, jt):
                ps = psm.tile([128, 1024], F32, tag="sb", bufs=2, name="pss")
                for e in range(2):
                    pb = 64 * e
                    nc.tensor.matmul(
                        ps[:, e * 512:(e + 1) * 512],
                        qk_sb[pb:pb + 64, 2 + p, jt * 128:(jt + 1) * 128],
                        qk_sb[pb:pb + 64, p, ic * 512 + 0:ic * 512 + 512],
                        start=True, stop=True,
                        tile_position=(pb, 0),
                    )
                return ps

            # AV runs one chunk behind its exp through p0 (pt tiles buffered
            # in the pP pool); the cascade closes at (1,0), which carries two
            # AV streams (the delayed (0,3) on o0/o1 and its own, same-slot,
            # on the otherwise-idle qk/vp banks). p1's remaining chunks are
            # same-slot as usual.
            PLAN = {(0, 0): [], (0, 1): [(0, 0)], (0, 2): [(0, 1)],
                    (0, 3): [(0, 2)], (1, 0): [(0, 3), (1, 0)],
                    (1, 1): [(1, 1)], (1, 2): [(1, 2)], (1, 3): [(1, 3)]}
            STAGS = [("o0", "o1"), ("qk", "vp")]
            seq = [(p, ic, jt)
                   for p in range(2) for ic in range(4) for jt in range(16)]
            ps_cur = s_group(*seq[0])
            pts = {}
            po_live = {}
            for idx, (p, ic, jt) in enumerate(seq):
                streams = PLAN[(p, ic)]
                if jt == 0:
                    for s, src in enumerate(streams):
                        po_live[src] = [ptile(STAGS[s][0]),
                                        ptile(STAGS[s][1])]
                pt = pP.tile([128, 1024], F16, tag="p")
                pts.setdefault((p, ic), []).append(pt)
                nc.scalar.activation(pt, ps_cur, EXP, scale=SCALE)
                # deferred work pops (before the AV matmuls of this slot)
                if (p, ic) in POPQ:
                    q, nf = POPQ[(p, ic)]
                    for _ in range(nf(jt)):
                        if q:
                            q.pop(0)()
                    if (ic, jt) == (3, 15):  # safety: all QKV before p1
                        for qq, _ in POPQ.values():
                            while qq:
                                qq.pop(0)()
                elif p == 1 and yq and 3 <= jt <= 13 and jt not in (8, 10, 12):
                    yq.pop(0)()
                if idx + 1 < len(seq):
                    ps_next = s_group(*seq[idx + 1])
                else:
                    ps_next = None
                for s, src in enumerate(streams):
                    po = po_live[src]
                    spt = pts[src][jt]
                    for e in range(2):
                        nc.tensor.matmul(
                            po[e][0:HD + 1, :],
                            v_aug[:, jt, 2 * src[0] + e, :],
                            spt[:, e * 512:(e + 1) * 512],
                            start=(jt == 0), stop=(jt == 15),
                        )
                ps_cur = ps_next
                if jt == 15:
                    for s, src in enumerate(streams):
                        fast = src == (1, 3)
                        if fast:
                            while yq:
                                yq.pop(0)()
                            psys = y_drain_open(3)
                        tmp = normalize(src[0], src[1], po_live.pop(src),
                                        tags=STAGS[s], fast=fast)
                        if src[0] == 1:
                            if src[1] < 3:
                                for k in range(8):
                                    yq.append(
                                        lambda it=4 * src[1] + k // 2,
                                        fc=k % 2: y_group(it, fc))
                            else:
                                y_drain_close(3, psys, tmp)


_NC = None


def _get_nc():
    global _NC
    if _NC is None:
        _NC = build_program()
    return _NC


def make_in_maps(x, w_qkv, w_out):
    x = np.asarray(x, dtype=np.float32)
    w_qkv = np.asarray(w_qkv, dtype=np.float32)
    w_out = np.asarray(w_out, dtype=np.float32)
    xT = [np.ascontiguousarray(x[b].T.astype(np.float16)) for b in range(B)]
    in_maps = []
    for c in range(NCORES):
        b, g = divmod(c, 4)
        f0 = g * NH * HD  # first feature col of this head group (256 wide)
        wq = w_qkv[:, f0:f0 + 256]
        wk = w_qkv[:, CDIM + f0:CDIM + f0 + 256]
        wv = w_qkv[:, 2 * CDIM + f0:2 * CDIM + f0 + 256]
        in_maps.append({
            "xT": xT[b],
            "wqkv": np.ascontiguousarray(np.concatenate(
                [wq[:, :128], wk[:, :128], wq[:, 128:], wk[:, 128:], wv],
                axis=1).astype(np.float16)),
            "wout": np.ascontiguousarray(
                w_out[f0:f0 + 256, :].astype(np.float16)),
        })
    return in_maps


def kernel(x, w_qkv, b_qkv, w_out, b_out, _trace=False):
    """Full inputs in, full (B, N, C) output out. b_qkv is all-zeros by the
    problem's input spec (fill: zeros); b_out is added on the host."""
    nc = _get_nc()
    in_maps = make_in_maps(x, w_qkv, w_out)
    res = run_bass_kernel_spmd(nc, in_maps, core_ids=list(range(NCORES)),
                               trace=_trace)
    out = np.zeros((B, NSEQ, CDIM), dtype=np.float32)
    for c in range(NCORES):
        out[c // 4] += res.results[c]["y"].astype(np.float32)
    out += np.asarray(b_out, dtype=np.float32)
    if _trace:
        kernel.last_exec_time_ns = res.exec_time_ns
        kernel.last_results = res
    return out


# revision 39
# speedup vs baseline: 1.1982x; 1.1982x over previous
"""Multi-head self-attention (B=2, N=2048, C=1024, H=16) on 8 TRN2 NeuronCores.

Sharding: data-parallel over batch (2) x tensor-parallel over heads (16/4=4 groups).
Core c handles batch b=c//4 and heads [4*(c%4), 4*(c%4)+4).

v3f (fp16 everywhere):
  - Inputs stream as a few wide strided DMAs ordered by need: q01/k01
    weights + the first quarter of x gate the first score (~16us), then V
    weights, the remaining x quarters, q23/k23 weights, w_out.
  - Wave-1 computes only q01[ic0] + k01[ic0] (the first-score gate) on one
    PSUM pair. Everything else in the QKV projection (16 V tiles, k01[ic1-3],
    q01[ic1-3], q23, k23) is split into small matmul units popped into PE
    idle slots under the ACT-saturated attention stream: 3 units/slot during
    chunk (p0,ic0) with deadlines v[nt] <= slot nt, k01[ic] <= slot 4*ic-1,
    then 1 unit/slot.
  - Attention per head pair: S^T = K^T.T @ Q^T, two heads packed in disjoint
    PE row groups (concurrent); P^T = exp(S*scale) on ACT (the pacing
    engine); O_aug^T = [V|1]^T @ P^T accumulated over key tiles (ones-column
    = softmax sums). The scores matmul for slot t+1 is emitted BEFORE the AV
    matmuls of slot t, so at chunk boundaries the next chunk's first exp
    follows the previous one back-to-back instead of waiting out the AV
    pipeline refill.
  - Normalize: sums row -> partition 0 via small SBUF DMA,
    reciprocal_approx_fast, gpsimd partition_broadcast, one DVE multiply per
    head (odd head shifts to partitions 64-127 via SBUF DMA).
  - Out-projection y_groups for chunk ic pop into chunk ic+1's slots; the
    final chunk's groups split their contraction (pair0 | pair1-even |
    pair1-odd) so matmuls overlap the normalize chain, with PSUM->SBUF
    copies alternating between DVE and the now-idle ACT; fp16 output.
Host sums the 4 per-batch partials (head groups) in fp32 and adds b_out.
"""

import contextlib

import numpy as np

import concourse.bass as bass
import concourse.bacc as bacc
import concourse.tile as tile
from concourse import library_config, mybir
from concourse.bass_utils import run_bass_kernel_spmd

B, NSEQ, CDIM, NHEADS, HD = 2, 2048, 1024, 16, 64
NH = 4          # heads per core
NCORES = 8
F32 = mybir.dt.float32
F16 = mybir.dt.float16
EXP = mybir.ActivationFunctionType.Exp
SCALE = HD ** -0.5


def build_program(dbg_probes=False):
    nc = bacc.Bacc("TRN2", target_bir_lowering=False, debug=False)

    xT = nc.dram_tensor("xT", [CDIM, NSEQ], F16, kind="ExternalInput").ap()
    wqkv = nc.dram_tensor("wqkv", [CDIM, 3 * NH * HD], F16, kind="ExternalInput").ap()
    wout = nc.dram_tensor("wout", [NH * HD, CDIM], F16, kind="ExternalInput").ap()
    y = nc.dram_tensor("y", [NSEQ, CDIM], F16, kind="ExternalOutput").ap()

    with tile.TileContext(nc) as tc:
        emit(nc, tc, xT, wqkv, wout, y)

    nc.compile()
    return nc


def emit(nc, tc, xT, wqkv, wout, y):
    ctx = contextlib.ExitStack()
    with ctx:
        const = ctx.enter_context(tc.tile_pool(name="const", bufs=1))

        # ---- persistent SBUF tensors ----
        wqkv_sb = const.tile([128, 8, 3 * NH * HD], F16)    # [p, ctile, 768]
        wout_sb = const.tile([128, 2, CDIM], F16)           # [p, ktile, 1024]
        xT_sb = const.tile([128, 8, NSEQ], F16)             # [p, ctile, 2048]
        qk_sb = const.tile([128, 4, NSEQ], F16)             # dim1: q01,q23,k01,k23
        v_aug = const.tile([128, 16, NH, HD + 1], F16)      # [p, ntile, head, V|1]
        o_sb = const.tile([128, 2, NSEQ], F16)              # normalized O^T, pairs
        r1t = const.tile([128, 4, 512], F32)                # sums/recip rows
        ones_b = const.tile([128, HD], F32)                 # fast-path bcast lhsT

        nc.gpsimd.load_library(library_config.attn)
        nc.vector.memset(v_aug[:, :, :, HD:HD + 1], 1.0)
        nc.vector.memset(ones_b, 1.0)

        # PSUM: 8 banks. sb (2 x [128,1024] = 4): wave-1 pair then scores
        # ping/pong. qk, vp: rolling accumulators for deferred QKV units and
        # out-proj psy. o0, o1: AV accumulators.
        with tc.tile_pool(name="pP", bufs=20) as pP, \
             tc.tile_pool(name="oup", bufs=2) as oup, \
             tc.tile_pool(name="rbc", bufs=4) as rbc, \
             tc.tile_pool(name="shf", bufs=2) as shf, \
             tc.tile_pool(name="yb", bufs=4) as yb, \
             tc.tile_pool(name="psm", bufs=1, space="PSUM") as psm:

            xT_t = xT.rearrange("(t p) n -> p t n", p=128)
            wqkv_t = wqkv.rearrange("(t p) f -> p t f", p=128)
            wout_t = wout.rearrange("(t p) f -> p t f", p=128)

            # -------- input DMA, ordered by first consumer -------------------
            # host wqkv col layout: [q01 | k01 | q23 | k23 | v]
            nc.sync.dma_start(wqkv_sb[:, :, 0:256], wqkv_t[:, :, 0:256])
            nc.sync.dma_start(xT_sb[:, 0:4, 0:512], xT_t[:, 0:4, 0:512])
            nc.sync.dma_start(xT_sb[:, 4:8, 0:512], xT_t[:, 4:8, 0:512])
            nc.sync.dma_start(wqkv_sb[:, :, 512:768], wqkv_t[:, :, 512:768])
            for cc in (1, 2, 3):
                nc.sync.dma_start(xT_sb[:, :, cc * 512:(cc + 1) * 512],
                                  xT_t[:, :, cc * 512:(cc + 1) * 512])
            nc.sync.dma_start(wqkv_sb[:, :, 256:512], wqkv_t[:, :, 256:512])
            for kt in range(2):
                nc.sync.dma_start(wout_sb[:, kt, :], wout_t[:, kt, :])

            TB = {"qk": 1, "vp": 1, "sb": 2, "o0": 1, "o1": 1}

            def ptile(tag, shape=(128, 512)):
                return psm.tile(list(shape), F32, tag=tag, bufs=TB[tag],
                                name=tag)

            # wqkv_sb column offset per feature group (host layout order)
            FT_COL = {0: 0, 2: 128, 1: 256, 3: 384}

            def qk_mms(ps, ft, ic, cts):
                c0 = FT_COL[ft]
                for ct in cts:
                    nc.tensor.matmul(
                        ps,
                        wqkv_sb[:, ct, c0:c0 + 128],
                        xT_sb[:, ct, ic * 512:(ic + 1) * 512],
                        start=(ct == 0), stop=(ct == 7),
                    )

            def v_mms(ps, nt, cts):
                for ct in cts:
                    nc.tensor.matmul(
                        ps[:, 0:256],
                        xT_sb[:, ct, nt * 128:(nt + 1) * 128],
                        wqkv_sb[:, ct, 512:768],
                        start=(ct == 0), stop=(ct == 7),
                    )

            def qk_evac(ps, ft, ic):
                nc.vector.tensor_copy(
                    qk_sb[:, ft, ic * 512:(ic + 1) * 512], ps)

            def v_evac(ps, nt):
                nc.vector.tensor_copy(v_aug[:, nt, :, 0:HD], ps[:, 0:256])

            # ---------------- QKV wave 1: the first-score gate ---------------
            sbA = ptile("sb", (128, 1024))
            for ct in range(8):
                qk_mms(sbA[:, 0:512], 0, 0, [ct])
                qk_mms(sbA[:, 512:1024], 2, 0, [ct])
            qk_evac(sbA[:, 0:512], 0, 0)
            qk_evac(sbA[:, 512:1024], 2, 0)

            # -------- deferred QKV chains as small matmul units --------------
            _tag_state = [0]

            def u_tag():
                _tag_state[0] ^= 1
                return "qk" if _tag_state[0] else "vp"

            def chain_units(kind, a):
                holder = {}
                parts = ([[0, 1], [2, 3], [4, 5], [6, 7]] if kind == "qk"
                         else [[0, 1, 2, 3], [4, 5, 6, 7]])

                def mk(cts, last):
                    def f():
                        if "tag" not in holder:
                            holder["tag"] = u_tag()
                        if cts[0] == 0:
                            holder["ps"] = ptile(holder["tag"])
                        ps = holder["ps"]
                        if kind == "qk":
                            qk_mms(ps, a[0], a[1], cts)
                            if last:
                                qk_evac(ps, a[0], a[1])
                        else:
                            v_mms(ps, a, cts)
                            if last:
                                v_evac(ps, a)
                    return f
                return [mk(cts, i == len(parts) - 1)
                        for i, cts in enumerate(parts)]

            V = {nt: chain_units("v", nt) for nt in range(16)}
            K1 = {ic: chain_units("qk", (2, ic)) for ic in (1, 2, 3)}
            Q01 = {ic: chain_units("qk", (0, ic)) for ic in (1, 2, 3)}
            Q23 = [chain_units("qk", (1, ic)) for ic in range(4)]
            K23 = [chain_units("qk", (3, ic)) for ic in range(4)]

            # Per-chunk pop queues. AV for chunk c runs one chunk later (pt
            # tiles are buffered), so (0,0) is scores+exp only and absorbs
            # k01[ic] (needed by slot 4*ic) plus half the V tiles; V[nt] is
            # only consumed once AV(0,0) runs during (0,1), slot nt.
            q00 = (K1[1][0:2] + [V[0][0], V[0][1]] + K1[1][2:4]
                   + [V[1][0], V[1][1]] + K1[2][0:2] + [V[2][0], V[2][1]]
                   + K1[2][2:4] + [V[3][0], V[3][1]] + K1[3][0:2]
                   + [V[4][0], V[4][1]] + K1[3][2:4] + [V[5][0], V[5][1]]
                   + Q01[1] + [V[6][0], V[6][1], V[7][0], V[7][1]])
            q01_ = ([u for nt in range(8, 16) for u in V[nt]] + Q01[2])
            q02 = (Q01[3] + [u for ch in Q23 for u in ch])
            q03 = [u for ch in K23 for u in ch]
            POPQ = {(0, 0): (q00, lambda jt: 2),
                    (0, 1): (q01_, lambda jt: 2 if jt < 4 else 1),
                    (0, 2): (q02, lambda jt: 2 if jt < 4 else 1),
                    (0, 3): (q03, lambda jt: 1)}

            yq = []  # deferred out-projection groups

            def y_group(it, fc):
                psy = ptile(u_tag())
                for pp in range(2):
                    nc.tensor.matmul(
                        psy,
                        o_sb[:, pp, it * 128:(it + 1) * 128],
                        wout_sb[:, pp, fc * 512:(fc + 1) * 512],
                        start=(pp == 0), stop=(pp == 1),
                    )
                y_sb = yb.tile([128, 512], F16, tag="ysb", name="ysbt")
                nc.vector.tensor_copy(y_sb, psy)
                nc.sync.dma_start(
                    y[it * 128:(it + 1) * 128, fc * 512:(fc + 1) * 512], y_sb)

            def y_drain(ic):
                # final-chunk out-projection: pair0 matmuls (start=True) run
                # during the normalize chain, the pair1 matmul joins once the
                # odd-head shift lands. Both are full-K so the PE serializes
                # them (disjoint-row-group matmuls would drain into the same
                # PSUM bank concurrently = collision). Copies alternate
                # DVE / (idle) ACT.
                for g, (it, fc) in enumerate(
                        (4 * ic + k // 2, k % 2) for k in range(8)):
                    psy = ptile(u_tag())
                    i1 = it * 128
                    f1 = fc * 512
                    nc.tensor.matmul(
                        psy, o_sb[:, 0, i1:i1 + 128],
                        wout_sb[:, 0, f1:f1 + 512], start=True, stop=False)
                    nc.tensor.matmul(
                        psy, o_sb[:, 1, i1:i1 + 128],
                        wout_sb[:, 1, f1:f1 + 512], start=False, stop=True)
                    y_sb = yb.tile([128, 512], F16, tag="ysb", name="ysbt")
                    if g % 2 == 0:
                        nc.vector.tensor_copy(y_sb, psy)
                    else:
                        nc.scalar.copy(y_sb, psy)
                    nc.sync.dma_start(y[i1:i1 + 128, f1:f1 + 512], y_sb)

            def normalize(p, ic, po, tags=("o0", "o1"), fast=False):
                i0 = ic * 512
                o_u = [oup.tile([HD + 1, 512], F32, tag=f"ou{e}",
                                name=f"ou{e}") for e in range(2)]
                # e1 chain first: its shift DMA is the longest pole
                nc.vector.tensor_copy(o_u[1], po[1][0:HD + 1, :])
                nc.vector.tensor_copy(o_u[0], po[0][0:HD + 1, :])
                rb = [None, None]
                if fast:
                    # tail path: broadcast the sums row with a small fp32 PE
                    # matmul into the just-freed po bank (PE is idle here and
                    # this keeps HAM warm), then reciprocal on 64 lanes
                    for e in (1, 0):
                        rbp = ptile(tags[e])
                        nc.tensor.matmul(
                            rbp[0:64, :], ones_b[HD:HD + 1, :],
                            o_u[e][HD:HD + 1, :], start=True, stop=True)
                        rb[e] = rbc.tile([64, 512], F32, tag="rb", name="rb")
                        nc.vector.reciprocal_approx_fast(rb[e], rbp[0:64, :])
                else:
                    r1 = [None, None]
                    for e in (1, 0):
                        r0 = r1t[0:1, 2 * e, :]
                        nc.sync.dma_start(r0, o_u[e][HD:HD + 1, :])
                        r1[e] = r1t[0:1, 2 * e + 1, :]
                        nc.vector.reciprocal_approx_fast(r1[e], r0)
                    for e in (1, 0):
                        rb[e] = rbc.tile([64, 512], F32, tag="rb", name="rb")
                        nc.gpsimd.partition_broadcast(rb[e], r1[e])
                tmp = shf.tile([64, 512], F16, tag="tmp")
                nc.vector.tensor_mul(tmp, o_u[1][0:64, :], rb[1])
                nc.sync.dma_start(o_sb[64:128, p, i0:i0 + 512], tmp)
                nc.vector.tensor_mul(
                    o_sb[0:64, p, i0:i0 + 512], o_u[0][0:64, :], rb[0])

            # ---------------- attention (flat, scores one slot ahead) --------
            def s_group(p, ic, jt):
                ps = psm.tile([128, 1024], F32, tag="sb", bufs=2, name="pss")
                for e in range(2):
                    pb = 64 * e
                    nc.tensor.matmul(
                        ps[:, e * 512:(e + 1) * 512],
                        qk_sb[pb:pb + 64, 2 + p, jt * 128:(jt + 1) * 128],
                        qk_sb[pb:pb + 64, p, ic * 512 + 0:ic * 512 + 512],
                        start=True, stop=True,
                        tile_position=(pb, 0),
                    )
                return ps

            # AV runs one chunk behind its exp through p0 (pt tiles buffered
            # in the pP pool); the cascade closes at (1,0), which carries two
            # AV streams (the delayed (0,3) on o0/o1 and its own, same-slot,
            # on the otherwise-idle qk/vp banks). p1's remaining chunks are
            # same-slot as usual.
            PLAN = {(0, 0): [], (0, 1): [(0, 0)], (0, 2): [(0, 1)],
                    (0, 3): [(0, 2)], (1, 0): [(0, 3), (1, 0)],
                    (1, 1): [(1, 1)], (1, 2): [(1, 2)], (1, 3): [(1, 3)]}
            STAGS = [("o0", "o1"), ("qk", "vp")]
            seq = [(p, ic, jt)
                   for p in range(2) for ic in range(4) for jt in range(16)]
            ps_cur = s_group(*seq[0])
            pts = {}
            po_live = {}
            for idx, (p, ic, jt) in enumerate(seq):
                streams = PLAN[(p, ic)]
                if jt == 0:
                    for s, src in enumerate(streams):
                        po_live[src] = [ptile(STAGS[s][0]),
                                        ptile(STAGS[s][1])]
                pt = pP.tile([128, 1024], F16, tag="p")
                pts.setdefault((p, ic), []).append(pt)
                nc.scalar.activation(pt, ps_cur, EXP, scale=SCALE)
                # deferred work pops (before the AV matmuls of this slot)
                if (p, ic) in POPQ:
                    q, nf = POPQ[(p, ic)]
                    for _ in range(nf(jt)):
                        if q:
                            q.pop(0)()
                    if (ic, jt) == (3, 15):  # safety: all QKV before p1
                        for qq, _ in POPQ.values():
                            while qq:
                                qq.pop(0)()
                elif p == 1 and yq and 3 <= jt <= 13 and jt not in (8, 10, 12):
                    yq.pop(0)()
                if idx + 1 < len(seq):
                    ps_next = s_group(*seq[idx + 1])
                else:
                    ps_next = None
                for s, src in enumerate(streams):
                    po = po_live[src]
                    spt = pts[src][jt]
                    for e in range(2):
                        nc.tensor.matmul(
                            po[e][0:HD + 1, :],
                            v_aug[:, jt, 2 * src[0] + e, :],
                            spt[:, e * 512:(e + 1) * 512],
                            start=(jt == 0), stop=(jt == 15),
                        )
                ps_cur = ps_next
                if jt == 15:
                    for s, src in enumerate(streams):
                        normalize(src[0], src[1], po_live.pop(src),
                                  tags=STAGS[s], fast=(src == (1, 3)))
                        if src[0] == 1:
                            if src[1] < 3:
                                for k in range(8):
                                    yq.append(
                                        lambda it=4 * src[1] + k // 2,
                                        fc=k % 2: y_group(it, fc))
                            else:
                                while yq:
                                    yq.pop(0)()
                                y_drain(3)


_NC = None


def _get_nc():
    global _NC
    if _NC is None:
        _NC = build_program()
    return _NC


def make_in_maps(x, w_qkv, w_out):
    x = np.asarray(x, dtype=np.float32)
    w_qkv = np.asarray(w_qkv, dtype=np.float32)
    w_out = np.asarray(w_out, dtype=np.float32)
    xT = [np.ascontiguousarray(x[b].T.astype(np.float16)) for b in range(B)]
    in_maps = []
    for c in range(NCORES):
        b, g = divmod(c, 4)
        f0 = g * NH * HD  # first feature col of this head group (256 wide)
        wq = w_qkv[:, f0:f0 + 256]
        wk = w_qkv[:, CDIM + f0:CDIM + f0 + 256]
        wv = w_qkv[:, 2 * CDIM + f0:2 * CDIM + f0 + 256]
        in_maps.append({
            "xT": xT[b],
            "wqkv": np.ascontiguousarray(np.concatenate(
                [wq[:, :128], wk[:, :128], wq[:, 128:], wk[:, 128:], wv],
                axis=1).astype(np.float16)),
            "wout": np.ascontiguousarray(
                w_out[f0:f0 + 256, :].astype(np.float16)),
        })
    return in_maps


def kernel(x, w_qkv, b_qkv, w_out, b_out, _trace=False):
    """Full inputs in, full (B, N, C) output out. b_qkv is all-zeros by the
    problem's input spec (fill: zeros); b_out is added on the host."""
    nc = _get_nc()
    in_maps = make_in_maps(x, w_qkv, w_out)
    res = run_bass_kernel_spmd(nc, in_maps, core_ids=list(range(NCORES)),
                               trace=_trace)
    out = np.zeros((B, NSEQ, CDIM), dtype=np.float32)
    for c in range(NCORES):
        out[c // 4] += res.results[c]["y"].astype(np.float32)
    out += np.asarray(b_out, dtype=np.float32)
    if _trace:
        kernel.last_exec_time_ns = res.exec_time_ns
        kernel.last_results = res
    return out
